# revision 25
# baseline (speedup 1.0000x reference)
"""Trainium2 Bass kernel for nn_BigramModel (unigram/bigram/trigram interpolated LM).

Strategy (pure data parallel, per sharding hint):
  - Shard text [256, 64] along batch dim across 8 cores -> [256, 8] each.
  - Replicate the tables on every core.
  - All tables in bf16 (tolerance is 2e-2; bf16 keeps us ~100x under it),
    which halves HBM traffic -> the kernel is DMA-bound at ~34 MB/core.
  - Host-side table prep folds the whole per-token elementwise pipeline away:
      q = 0.75*uni + bi + 0.75*tri  (everything scaled by 1/ALPHA folds out
      in the normalization), so we store  aug = bigram + 0.75*uni  with an
      extra column holding the row sum.  Then per token:
        gather aug row (bf16), DMA-accumulate the (pre-scaled 0.75*) trigram
        row into it on a trigram hit (~0.1% of tokens; OOB-skipped misses
        cost a 4-byte null packet, no HBM read), z = sumcol + 0.75*hit,
        out = Ln(q/z + EPS) on the scalar engine, DMA out in bf16.
    The vector engine only touches [128,1]/[128,8] minis; the scalar engine
    (Ln) runs ~59us/core; both hide under the ~95us DMA floor.
"""

import numpy as np
import ml_dtypes

import concourse.bass as bass
import concourse.bacc as bacc
import concourse.tile as tile
from concourse import mybir
from concourse.bass_utils import run_bass_kernel_spmd

V = 4096
S = 256
B = 64
K = 20000
NCORES = 8
BS = B // NCORES  # 8 batch columns per core
P = 128
VA = V + 2  # augmented row: [0:V] = bigram + 0.75*uni, [V] = row sum, [V+1] = pad

ALPHA = 0.4
BETA = 0.3
C1 = 1.0 - ALPHA - BETA  # 0.3
R_UNI = C1 / ALPHA  # 0.75
R_TRI = BETA / ALPHA  # 0.75
EPS = 1e-10

f32 = mybir.dt.float32
bf16 = mybir.dt.bfloat16
i32 = mybir.dt.int32
BF16 = ml_dtypes.bfloat16


def build_nc(n_b: int = BS, k_miss: int | None = None) -> bass.Bass:
    nc = bacc.Bacc("TRN2", num_devices=NCORES)

    text = nc.dram_tensor("text", [S, n_b], i32, kind="ExternalInput")
    bigram = nc.dram_tensor("bigram_table", [V, VA], bf16, kind="ExternalInput")
    tri_rows = nc.dram_tensor("tri_rows", [K, V], bf16, kind="ExternalInput")
    tri_map = nc.dram_tensor("tri_map", [V * V, 1], i32, kind="ExternalInput")
    out = nc.dram_tensor("out", [S, n_b * V], bf16, kind="ExternalOutput")

    n_h = S // P  # seq halves (2)
    n_tiles = n_b * n_h
    LOOKAHEAD = 3
    GBUFS = LOOKAHEAD + 3

    with tile.TileContext(nc) as tc:
        with (
            tc.tile_pool(name="const", bufs=1) as const_pool,
            tc.tile_pool(name="half", bufs=n_h) as half,
            tc.tile_pool(name="bi", bufs=GBUFS) as bi_pool,
            tc.tile_pool(name="tri", bufs=GBUFS) as tri_pool,
            tc.tile_pool(name="ot", bufs=4) as out_pool,
            tc.tile_pool(name="small", bufs=n_tiles) as small,
        ):
            eps_b = const_pool.tile([P, 1], f32, tag="eps_b")
            nc.vector.memset(eps_b[:], EPS)

            # ---- phase 1: per-half (128 x n_b) token index prep ----
            curs, fks = [], []
            for h in range(n_h):
                s0 = h * P
                cur = half.tile([P, n_b], i32, tag="cur")
                nc.sync.dma_start(cur[:], text[s0 : s0 + P, :])
                prv = half.tile([P, n_b], i32, tag="prv")
                if h == 0:
                    nc.sync.dma_start(prv[0:1, :], text[0:1, :])
                    nc.sync.dma_start(prv[1:P, :], text[0 : P - 1, :])
                else:
                    nc.sync.dma_start(prv[:], text[s0 - 1 : s0 + P - 1, :])

                # flat trigram key = prev * 4096 + cur (exact, < 2^24)
                fk = half.tile([P, n_b], i32, tag="fk")
                nc.vector.scalar_tensor_tensor(
                    out=fk[:],
                    in0=prv[:],
                    scalar=V,
                    in1=cur[:],
                    op0=mybir.AluOpType.mult,
                    op1=mybir.AluOpType.add,
                )
                if h == 0 and k_miss is not None:
                    # seq positions 0,1 never take the trigram branch: remap
                    # their keys to a known tri_map miss
                    nc.vector.memset(fk[0:2, :], k_miss)
                curs.append(cur)
                fks.append(fk)

            # ---- phase 2: software-pipelined per-tile work ----
            # gpsimd (SWDGE Q7 descriptor emission + ring backpressure) and
            # HBM bytes are the scarce resources: keep the gpsimd queue free
            # of data-dependent stalls by running index prep LOOKAHEAD tiles
            # ahead, and keep the combine on the vector engine in cheap 2x
            # ops (tri tile zeroed, plain tensor add).
            tiles = [(b, h) for b in range(n_b) for h in range(n_h)]
            risks, hits, tris = {}, {}, {}

            def issue_prep(t):
                b, h = tiles[t]
                ridx = small.tile([P, 1], i32, tag="ridx")
                nc.gpsimd.indirect_dma_start(
                    out=ridx[:],
                    out_offset=None,
                    in_=tri_map[:],
                    in_offset=bass.IndirectOffsetOnAxis(
                        ap=fks[h][:, b : b + 1], axis=0
                    ),
                )
                if h == 0 and k_miss is None:
                    nc.vector.memset(ridx[0:2, :], -1)
                # miss (-1) -> 65535 which fails bounds_check -> skipped
                risk = small.tile([P, 1], i32, tag="risk")
                nc.vector.tensor_scalar(
                    out=risk[:],
                    in0=ridx[:],
                    scalar1=0xFFFF,
                    scalar2=None,
                    op0=mybir.AluOpType.bitwise_and,
                )
                # hit indicator in {0.0, 1.0}
                hit = small.tile([P, 1], f32, tag="hit")
                nc.vector.tensor_scalar(
                    out=hit[:],
                    in0=ridx[:],
                    scalar1=0,
                    scalar2=None,
                    op0=mybir.AluOpType.is_ge,
                )
                tri = tri_pool.tile([P, V], bf16, tag="tri")
                if t < GBUFS:
                    # first touch of each slot: clear so skipped rows stay
                    # finite (afterwards stale data is old tri rows, which
                    # the hit mask zeroes)
                    nc.vector.memset(tri[:], 0.0)
                risks[t] = risk
                hits[t] = hit
                tris[t] = tri

            def issue_main(t):
                b, h = tiles[t]
                s0 = h * P
                cur, risk, hit, tri = curs[h], risks[t], hits[t], tris[t]

                bi = bi_pool.tile([P, VA], bf16, tag="bi")
                nc.gpsimd.indirect_dma_start(
                    out=bi[:],
                    out_offset=None,
                    in_=bigram[:],
                    in_offset=bass.IndirectOffsetOnAxis(
                        ap=cur[:, b : b + 1], axis=0
                    ),
                )
                nc.gpsimd.indirect_dma_start(
                    out=tri[:],
                    out_offset=None,
                    in_=tri_rows[:],
                    in_offset=bass.IndirectOffsetOnAxis(ap=risk[:, :1], axis=0),
                    bounds_check=K - 1,
                    oob_is_err=False,
                )

                # z = rowsum + 0.75*hit (sum col; EPS/ALPHA = 2.5e-10 is
                # below f32/bf16 resolution of z ~ 1.75 so reference's +EPS
                # in the denominator is a no-op here)
                z = small.tile([P, 1], f32, tag="z")
                nc.vector.scalar_tensor_tensor(
                    out=z[:],
                    in0=hit[:, :1],
                    scalar=R_TRI,
                    in1=bi[:, V : V + 1],
                    op0=mybir.AluOpType.mult,
                    op1=mybir.AluOpType.add,
                )
                r = small.tile([P, 1], f32, tag="r")
                nc.vector.reciprocal(r[:], z[:])

                # q = tri*hit + bi (tri pre-scaled by 0.75; hit masks stale
                # rows from skipped gathers)
                nc.vector.scalar_tensor_tensor(
                    out=bi[:, 0:V],
                    in0=tri[:],
                    scalar=hit[:, :1],
                    in1=bi[:, 0:V],
                    op0=mybir.AluOpType.mult,
                    op1=mybir.AluOpType.add,
                )

                ot = out_pool.tile([P, V], bf16, tag="ot")
                nc.scalar.activation(
                    out=ot[:],
                    in_=bi[:, 0:V],
                    func=mybir.ActivationFunctionType.Ln,
                    bias=eps_b[:, :1],
                    scale=r[:, :1],
                )

                nc.sync.dma_start(out[s0 : s0 + P, b * V : (b + 1) * V], ot[:])

            for t in range(min(LOOKAHEAD, n_tiles)):
                issue_prep(t)
            for t in range(n_tiles):
                # main before prep: the ops feeding ACT(t) go ahead of the
                # lookahead prep ops in each engine queue
                issue_main(t)
                if t + LOOKAHEAD < n_tiles:
                    issue_prep(t + LOOKAHEAD)

    nc.finalize()
    return nc


def _prep_inputs(text, unigram, bigram_table, tri_rows, tri_map):
    """Shared (replicated) device arrays, keyed by dram tensor name."""
    text = np.ascontiguousarray(np.asarray(text, dtype=np.int32))
    uni = np.asarray(unigram, np.float32).reshape(1, V)
    bt = np.asarray(bigram_table, np.float32) + R_UNI * uni  # fold unigram in
    aug = np.zeros((V, VA), dtype=BF16)
    aug[:, :V] = bt.astype(BF16)
    aug[:, V] = bt.sum(axis=1).astype(BF16)
    tr = np.ascontiguousarray(
        (np.asarray(tri_rows, np.float32) * R_TRI).astype(BF16)
    )
    tm = np.ascontiguousarray(np.asarray(tri_map, np.int32).reshape(V * V, 1))
    shared = {
        "text": text,
        "bigram_table": np.ascontiguousarray(aug),
        "tri_rows": np.ascontiguousarray(tr),
        "tri_map": tm,
    }
    # any key absent from the trigram dict (used to force seq pos 0,1 to miss)
    k_miss = int(np.flatnonzero(tm[:, 0] < 0)[0])
    return shared, k_miss


def make_in_maps(shared):
    text = shared["text"]
    in_maps = []
    for c in range(NCORES):
        m = dict(shared)
        m["text"] = np.ascontiguousarray(text[:, c * BS : (c + 1) * BS])
        in_maps.append(m)
    return in_maps


def kernel(text, unigram, bigram_table, tri_rows, tri_map, _trace=False, _trace_kwargs=None):
    shared, k_miss = _prep_inputs(text, unigram, bigram_table, tri_rows, tri_map)
    nc = build_nc(BS, k_miss=k_miss)
    in_maps = make_in_maps(shared)
    res = run_bass_kernel_spmd(
        nc,
        in_maps,
        core_ids=list(range(NCORES)),
        trace=_trace,
        **(_trace_kwargs or {}),
    )
    outs = [
        np.asarray(res.results[c]["out"]).astype(np.float32).reshape(S, BS, V)
        for c in range(NCORES)
    ]
    full = np.concatenate(outs, axis=1)
    if _trace:
        return full, res
    return full


# revision 26
# speedup vs baseline: 1.1497x; 1.1497x over previous
"""Trainium2 Bass kernel for nn_BigramModel (unigram/bigram/trigram interpolated LM).

Strategy (pure data parallel, per sharding hint):
  - Shard text [256, 64] along batch dim across 8 cores -> [256, 8] each.
  - Replicate the tables on every core.
  - All tables in bf16 (tolerance is 2e-2; bf16 keeps us ~100x under it),
    which halves HBM traffic -> the kernel is DMA-bound at ~34 MB/core.
  - Host-side table prep folds the whole per-token elementwise pipeline away:
      q = 0.75*uni + bi + 0.75*tri  (everything scaled by 1/ALPHA folds out
      in the normalization), so we store  aug = bigram + 0.75*uni  with an
      extra column holding the row sum.  Then per token:
        gather aug row (bf16), DMA-accumulate the (pre-scaled 0.75*) trigram
        row into it on a trigram hit (~0.1% of tokens; OOB-skipped misses
        cost a 4-byte null packet, no HBM read), z = sumcol + 0.75*hit,
        out = Ln(q/z + EPS) on the scalar engine, DMA out in bf16.
    The vector engine only touches [128,1]/[128,8] minis; the scalar engine
    (Ln) runs ~59us/core; both hide under the ~95us DMA floor.
"""

import numpy as np
import ml_dtypes

import concourse.bass as bass
import concourse.bacc as bacc
import concourse.tile as tile
from concourse import mybir
from concourse.bass_utils import run_bass_kernel_spmd

V = 4096
S = 256
B = 64
K = 20000
NCORES = 8
BS = B // NCORES  # 8 batch columns per core
P = 128
VA = V + 2  # augmented row: [0:V] = bigram + 0.75*uni, [V] = row sum, [V+1] = pad

ALPHA = 0.4
BETA = 0.3
C1 = 1.0 - ALPHA - BETA  # 0.3
R_UNI = C1 / ALPHA  # 0.75
R_TRI = BETA / ALPHA  # 0.75
EPS = 1e-10

f32 = mybir.dt.float32
bf16 = mybir.dt.bfloat16
i32 = mybir.dt.int32
BF16 = ml_dtypes.bfloat16


def build_nc(n_b: int = BS, k_miss: int | None = None) -> bass.Bass:
    nc = bacc.Bacc("TRN2", num_devices=NCORES)

    text = nc.dram_tensor("text", [S, n_b], i32, kind="ExternalInput")
    bigram = nc.dram_tensor("bigram_table", [V, VA], bf16, kind="ExternalInput")
    tri_rows = nc.dram_tensor("tri_rows", [K, V], bf16, kind="ExternalInput")
    tri_map = nc.dram_tensor("tri_map", [V * V, 1], i32, kind="ExternalInput")
    out = nc.dram_tensor("out", [S, n_b * V], bf16, kind="ExternalOutput")

    n_h = S // P  # seq halves (2)
    n_tiles = n_b * n_h
    LOOKAHEAD = 3
    GBUFS = LOOKAHEAD + 4

    with tile.TileContext(nc) as tc:
        with (
            tc.tile_pool(name="const", bufs=1) as const_pool,
            tc.tile_pool(name="half", bufs=n_h) as half,
            tc.tile_pool(name="bi", bufs=GBUFS) as bi_pool,
            tc.tile_pool(name="tri", bufs=GBUFS) as tri_pool,
            tc.tile_pool(name="ot", bufs=4) as out_pool,
            tc.tile_pool(name="small", bufs=n_tiles) as small,
        ):
            eps_b = const_pool.tile([P, 1], f32, tag="eps_b")
            nc.vector.memset(eps_b[:], EPS)

            # ---- phase 1: per-half (128 x n_b) token index prep ----
            curs, fks = [], []
            for h in range(n_h):
                s0 = h * P
                cur = half.tile([P, n_b], i32, tag="cur")
                nc.sync.dma_start(cur[:], text[s0 : s0 + P, :])
                prv = half.tile([P, n_b], i32, tag="prv")
                if h == 0:
                    nc.sync.dma_start(prv[0:1, :], text[0:1, :])
                    nc.sync.dma_start(prv[1:P, :], text[0 : P - 1, :])
                else:
                    nc.sync.dma_start(prv[:], text[s0 - 1 : s0 + P - 1, :])

                # flat trigram key = prev * 4096 + cur (exact, < 2^24)
                fk = half.tile([P, n_b], i32, tag="fk")
                nc.vector.scalar_tensor_tensor(
                    out=fk[:],
                    in0=prv[:],
                    scalar=V,
                    in1=cur[:],
                    op0=mybir.AluOpType.mult,
                    op1=mybir.AluOpType.add,
                )
                if h == 0 and k_miss is not None:
                    # seq positions 0,1 never take the trigram branch: remap
                    # their keys to a known tri_map miss
                    nc.vector.memset(fk[0:2, :], k_miss)
                curs.append(cur)
                fks.append(fk)

            # ---- phase 2: software-pipelined per-tile work ----
            # gpsimd (SWDGE Q7 descriptor emission + ring backpressure) and
            # HBM bytes are the scarce resources: keep the gpsimd queue free
            # of data-dependent stalls by running index prep LOOKAHEAD tiles
            # ahead, and keep the combine on the vector engine in cheap 2x
            # ops (tri tile zeroed, plain tensor add).
            tiles = [(b, h) for b in range(n_b) for h in range(n_h)]
            risks, hits, tris = {}, {}, {}

            def issue_prep(t):
                b, h = tiles[t]
                ridx = small.tile([P, 1], i32, tag="ridx")
                nc.gpsimd.indirect_dma_start(
                    out=ridx[:],
                    out_offset=None,
                    in_=tri_map[:],
                    in_offset=bass.IndirectOffsetOnAxis(
                        ap=fks[h][:, b : b + 1], axis=0
                    ),
                )
                if h == 0 and k_miss is None:
                    nc.vector.memset(ridx[0:2, :], -1)
                # miss (-1) -> 65535 which fails bounds_check -> skipped
                risk = small.tile([P, 1], i32, tag="risk")
                nc.vector.tensor_scalar(
                    out=risk[:],
                    in0=ridx[:],
                    scalar1=0xFFFF,
                    scalar2=None,
                    op0=mybir.AluOpType.bitwise_and,
                )
                # hit indicator in {0.0, 1.0}
                hit = small.tile([P, 1], f32, tag="hit")
                nc.vector.tensor_scalar(
                    out=hit[:],
                    in0=ridx[:],
                    scalar1=0,
                    scalar2=None,
                    op0=mybir.AluOpType.is_ge,
                )
                # zero the tri tile ahead of time so skipped (miss) rows
                # contribute exactly 0 to the add
                tri = tri_pool.tile([P, V], bf16, tag="tri")
                nc.vector.memset(tri[:], 0.0)
                risks[t] = risk
                hits[t] = hit
                tris[t] = tri

            bis = {}

            def issue_gather(t):
                b, h = tiles[t]
                cur, risk = curs[h], risks[t]
                bi = bi_pool.tile([P, VA], bf16, tag="bi")
                nc.gpsimd.indirect_dma_start(
                    out=bi[:],
                    out_offset=None,
                    in_=bigram[:],
                    in_offset=bass.IndirectOffsetOnAxis(
                        ap=cur[:, b : b + 1], axis=0
                    ),
                )
                nc.gpsimd.indirect_dma_start(
                    out=tris[t][:],
                    out_offset=None,
                    in_=tri_rows[:],
                    in_offset=bass.IndirectOffsetOnAxis(ap=risk[:, :1], axis=0),
                    bounds_check=K - 1,
                    oob_is_err=False,
                )
                bis[t] = bi

            def issue_compute(t):
                b, h = tiles[t]
                s0 = h * P
                bi, hit, tri = bis[t], hits[t], tris[t]

                # z = rowsum + 0.75*hit (sum col; EPS/ALPHA = 2.5e-10 is
                # below f32/bf16 resolution of z ~ 1.75 so reference's +EPS
                # in the denominator is a no-op here)
                z = small.tile([P, 1], f32, tag="z")
                nc.vector.scalar_tensor_tensor(
                    out=z[:],
                    in0=hit[:, :1],
                    scalar=R_TRI,
                    in1=bi[:, V : V + 1],
                    op0=mybir.AluOpType.mult,
                    op1=mybir.AluOpType.add,
                )
                r = small.tile([P, 1], f32, tag="r")
                nc.vector.reciprocal(r[:], z[:])

                # q = tri + bi (tri pre-scaled by 0.75, zero on miss; 2x TT)
                nc.vector.tensor_tensor(
                    out=bi[:, 0:V],
                    in0=tri[:],
                    in1=bi[:, 0:V],
                    op=mybir.AluOpType.add,
                )

                ot = out_pool.tile([P, V], bf16, tag="ot")
                nc.scalar.activation(
                    out=ot[:],
                    in_=bi[:, 0:V],
                    func=mybir.ActivationFunctionType.Ln,
                    bias=eps_b[:, :1],
                    scale=r[:, :1],
                )

                nc.sync.dma_start(out[s0 : s0 + P, b * V : (b + 1) * V], ot[:])

            for t in range(min(LOOKAHEAD, n_tiles)):
                issue_prep(t)
            # compute runs one iteration behind the gathers so vector/scalar
            # ops never head-of-line-wait on a just-issued DMA
            for t in range(n_tiles + 1):
                if t < n_tiles:
                    issue_gather(t)
                if t >= 1:
                    issue_compute(t - 1)
                if t + LOOKAHEAD < n_tiles:
                    issue_prep(t + LOOKAHEAD)

    nc.finalize()
    return nc


def _prep_inputs(text, unigram, bigram_table, tri_rows, tri_map):
    """Shared (replicated) device arrays, keyed by dram tensor name."""
    text = np.ascontiguousarray(np.asarray(text, dtype=np.int32))
    uni = np.asarray(unigram, np.float32).reshape(1, V)
    bt = np.asarray(bigram_table, np.float32) + R_UNI * uni  # fold unigram in
    aug = np.zeros((V, VA), dtype=BF16)
    aug[:, :V] = bt.astype(BF16)
    aug[:, V] = bt.sum(axis=1).astype(BF16)
    tr = np.ascontiguousarray(
        (np.asarray(tri_rows, np.float32) * R_TRI).astype(BF16)
    )
    tm = np.ascontiguousarray(np.asarray(tri_map, np.int32).reshape(V * V, 1))
    shared = {
        "text": text,
        "bigram_table": np.ascontiguousarray(aug),
        "tri_rows": np.ascontiguousarray(tr),
        "tri_map": tm,
    }
    # any key absent from the trigram dict (used to force seq pos 0,1 to miss)
    k_miss = int(np.flatnonzero(tm[:, 0] < 0)[0])
    return shared, k_miss


def make_in_maps(shared):
    text = shared["text"]
    in_maps = []
    for c in range(NCORES):
        m = dict(shared)
        m["text"] = np.ascontiguousarray(text[:, c * BS : (c + 1) * BS])
        in_maps.append(m)
    return in_maps


def kernel(text, unigram, bigram_table, tri_rows, tri_map, _trace=False, _trace_kwargs=None):
    shared, k_miss = _prep_inputs(text, unigram, bigram_table, tri_rows, tri_map)
    nc = build_nc(BS, k_miss=k_miss)
    in_maps = make_in_maps(shared)
    res = run_bass_kernel_spmd(
        nc,
        in_maps,
        core_ids=list(range(NCORES)),
        trace=_trace,
        **(_trace_kwargs or {}),
    )
    outs = [
        np.asarray(res.results[c]["out"]).astype(np.float32).reshape(S, BS, V)
        for c in range(NCORES)
    ]
    full = np.concatenate(outs, axis=1)
    if _trace:
        return full, res
    return full
